# revision 34
# baseline (speedup 1.0000x reference)
"""Multi-head causal attention (B=2, S=2048, D=1024, H=16) on 8 TRN2 cores.

Sharding: tensor-parallel over heads. Core c owns heads {2c, 2c+1} and rows
[128c, 128c+128) of Wo. Each core computes its heads' attention and the
partial output projection; the host sums the 8 partials (the "all-reduce")
and adds the bias.

Device layout (all bf16 in SBUF, f32 PSUM accumulation):
  xT      [1024, 4096]  x transposed: xT[d, b*2048+s] = x[b,s,d]
  wq/wk/wv [1024, 128]  two heads' weights packed on columns
  wo      [128, 1024]   Wo rows for this core
  out_pT  [1024, 4096]  partial^T: out_pT[d, b*2048+s]

Schedule per core:
  1. DMA priority: wq/wk, then batch-0 first-quarter columns of every xT
     d-chunk, then the rest of batch 0, then wv/wo, then batch 1. Phase 1a
     projects Q^T/K^T for batch-0 columns 0:1024 only (DMA-paced); the
     remaining projection chunks (batch-0 second half + batch 1) run as
     4-matmul filler ticks inside the attention loop.
  2. Causal attention per batch with BOTH local heads together. Each kj
     step emits only the (s_lo, 1024) score piece during kj<8; the
     (1024, 2048) pieces are deferred to steps 8..15 (paired with those
     steps' single pieces), so attention starts as soon as phase 1a ends.
     Score matmuls are K=64 and the two heads' kT slices sit at partitions
     0:64 / 64:128 -> different PE row groups; with a 3-deep ps ring both
     heads' matmuls are ready together and stream concurrently. exp on ACT
     (the phase pacer); scores for step kj+1 are emitted BEFORE step kj's
     AV-burst/filler backlog so ACT never starves behind filler streams.
  3. Quarter-major deferred AV per head (one PSUM bank per quarter burst,
     delayed one extra step for pipeline slack); normalization = fast DVE
     reciprocal + bf16 K=1 broadcast matmul + one DVE multiply into OT.
  4. Partial projection out_pT = wo_rows.T @ OT paced as PE filler during
     batch 1's attention; 4-dc half-stages DMA out as soon as they cast.
"""

import numpy as np
import ml_dtypes

B, S, D, H = 2, 2048, 1024, 16
HD = 64          # head dim
NCORES = 8
HL = H // NCORES  # local heads per core = 2
BS = B * S        # 4096
SCALE = float(D) ** -0.5

BF16 = ml_dtypes.bfloat16

_CACHE = {}


def _build_kernel():
    import concourse.mybir as mybir
    import concourse.tile as tile
    from concourse import bacc

    bf16 = mybir.dt.bfloat16
    f32 = mybir.dt.float32
    Exp = mybir.ActivationFunctionType.Exp

    nc = bacc.Bacc("TRN2", debug=False, enable_asserts=False)
    xT_d = nc.dram_tensor("xT", [D, BS], bf16, kind="ExternalInput").ap()
    wq_d = nc.dram_tensor("wq", [D, 128], bf16, kind="ExternalInput").ap()
    wk_d = nc.dram_tensor("wk", [D, 128], bf16, kind="ExternalInput").ap()
    wv_d = nc.dram_tensor("wv", [D, 128], bf16, kind="ExternalInput").ap()
    wo_d = nc.dram_tensor("wo", [128, D], bf16, kind="ExternalInput").ap()
    # consts: cols 0:128 = upper-tri mask (1 where col >= row); cols 128:192
    # unused (kept for host-side layout compat).
    consts_d = nc.dram_tensor("consts", [128, 192], bf16, kind="ExternalInput").ap()
    out_d = nc.dram_tensor("out_pT", [D, BS], bf16, kind="ExternalOutput").ap()

    DC = D // 128   # 8 d-chunks
    NT = S // 128   # 16 key blocks per sequence

    with tile.TileContext(nc) as tc:
        with tc.tile_pool(name="persist", bufs=1) as pp:
            xT = pp.tile([128, DC, BS], bf16, tag="xT")
            qT = pp.tile([128, BS], bf16, tag="qT")
            kT = pp.tile([128, BS], bf16, tag="kT")
            # V in [t, k] layout, padded to 128 columns: col 0 = 1.0 (makes
            # the attention matmul emit softmax denominators in PSUM
            # partition 0), cols 1:64 = 0, cols 64:128 = V block for s-block
            # g (g = 16*b + t16) and local head j. The V block starts at 64
            # so the 64 numerator rows sit at a size-aligned PSUM offset.
            V_sb = pp.tile([128, BS // 128, HL, 128], bf16, tag="V")
            OT = pp.tile([128, BS], bf16, tag="OT")
            wq = pp.tile([128, DC, 128], bf16, tag="wq")
            wk = pp.tile([128, DC, 128], bf16, tag="wk")
            wv = pp.tile([128, DC, 128], bf16, tag="wv")
            wo = pp.tile([128, D], bf16, tag="wo")
            consts = pp.tile([128, 192], bf16, tag="consts")
            trimask = consts[:, 0:128]
            ones64 = pp.tile([1, 64], bf16, tag="ones64")

            nc.sync.dma_start(consts[:], consts_d[:])
            nc.vector.memset(ones64[:], 1.0)
            # Preheat the ACT exp table so the first real exp doesn't pay
            # the table-load latency mid-pipeline.
            warmup = pp.tile([1, 8], bf16, tag="warmup")
            nc.scalar.activation(warmup[:], consts[0:1, 0:8], Exp, scale=SCALE)
            nc.vector.memset(V_sb[:, :, :, 0:HD], 0.0)
            nc.vector.memset(V_sb[:, :, :, 0:1], 1.0)

            # DMA priority order (see module docstring).
            xT_r = xT_d.rearrange("(o p) s -> p o s", p=128)
            nc.sync.dma_start(wq[:], wq_d.rearrange("(o p) c -> p o c", p=128))
            nc.sync.dma_start(wk[:], wk_d.rearrange("(o p) c -> p o c", p=128))
            # per-chunk first-quarter DMAs spread across queues so phase 1a
            # can start on chunk 0 while later chunks stream
            for o in range(DC):
                nc.sync.dma_start(xT[:, o, 0:1024], xT_r[:, o, 0:1024])
            for o in range(0, DC, 4):
                nc.sync.dma_start(xT[:, o : o + 4, 1024:2048],
                                  xT_r[:, o : o + 4, 1024:2048])
            nc.sync.dma_start(wv[:], wv_d.rearrange("(o p) c -> p o c", p=128))
            nc.sync.dma_start(wo[:], wo_d[:])
            for o in range(0, DC, 4):
                nc.sync.dma_start(xT[:, o : o + 4, S:BS],
                                  xT_r[:, o : o + 4, S:BS])

            # ---- Phase 1a: Q^T/K^T for batch-0 columns 0:1024 ----
            # The preheat pool coexists with phase 1a's banks (no WAR), so
            # ~32 dummy matmuls run in phase 1a's DMA-wait gaps and flip
            # the HAM clock gate to 8/8 before the real stream arrives.
            with (
                tc.tile_pool(name="preheat", bufs=1, space="PSUM") as php,
                tc.tile_pool(name="ph1psum", bufs=4, space="PSUM") as ph1,
            ):
                pre = php.tile([128, 512], f32, tag="pre")
                for _ in range(32):
                    nc.tensor.matmul(
                        pre[:, 0:128], lhsT=consts[:, 0:128],
                        rhs=consts[:, 0:128], start=True, stop=True,
                    )
                ps1 = {}
                for di in range(2):
                    for sc in range(2):
                        ps1[(di, sc)] = ph1.tile([128, 512], f32, tag="ph1",
                                                 name=f"ph1_{di}_{sc}")
                for o in range(DC):
                    for di, w_sb in ((0, wq), (1, wk)):
                        for sc in range(2):
                            nc.tensor.matmul(
                                ps1[(di, sc)][:],
                                lhsT=w_sb[:, o, :],
                                rhs=xT[:, o, 512 * sc : 512 * (sc + 1)],
                                start=(o == 0),
                                stop=(o == DC - 1),
                            )
                # copy order q-sc0, k-sc0 first (unblocks the first score
                # matmul), alternating DVE/ACT so the casts overlap
                for n, (di, sc) in enumerate([(0, 0), (1, 0), (0, 1), (1, 1)]):
                    dst = qT if di == 0 else kT
                    t = ps1[(di, sc)]
                    if n % 2 == 0:
                        nc.vector.tensor_copy(dst[:, 512 * sc : 512 * (sc + 1)],
                                              t[:])
                    else:
                        nc.scalar.copy(dst[:, 512 * sc : 512 * (sc + 1)], t[:])

            # ---- Phase 2+3: paired-head causal attention ----
            with (
                tc.tile_pool(name="ps", bufs=3, space="PSUM") as ps_pool,
                tc.tile_pool(name="shared", bufs=2, space="PSUM") as sh_pool,
                tc.tile_pool(name="expp", bufs=36) as exp_pool,
                tc.tile_pool(name="recip", bufs=3) as rc_pool,
                tc.tile_pool(name="onum", bufs=3) as on_pool,
                tc.tile_pool(name="ph4out", bufs=2) as ph4o,
            ):
                def emit_piece_pair(b, kj, p0, p1, ets):
                    # One score piece (columns p0:p1) + exp for BOTH heads.
                    # Head j's matmuls use kT partitions 64j:64j+64 -> PE
                    # row group j; with the 3-deep ps ring both heads'
                    # matmuls are ready together and stream concurrently.
                    s_lo = 128 * kj
                    w = p1 - p0
                    psj = [ps_pool.tile([128, 1024], f32, tag="ps", name="ps")
                           for _ in range(HL)]
                    for c0 in range(0, w, 512):
                        c1 = min(c0 + 512, w)
                        for j in range(HL):
                            kTh = kT[64 * j : 64 * (j + 1), S * b : S * (b + 1)]
                            qTh = qT[64 * j : 64 * (j + 1), S * b : S * (b + 1)]
                            nc.tensor.matmul(
                                psj[j][:, c0:c1],
                                lhsT=kTh[:, s_lo : s_lo + 128],
                                rhs=qTh[:, p0 + c0 : p0 + c1],
                                start=True,
                                stop=True,
                            )
                    for j in range(HL):
                        et = exp_pool.tile([128, 1024], bf16, tag="expT",
                                           name="et")
                        nc.scalar.activation(et[:, 0:w], psj[j][:, 0:w],
                                             Exp, scale=SCALE)
                        if p0 == s_lo:
                            # diagonal 128x128: keep only s' >= t. On
                            # GpSimd (otherwise idle): both operands are
                            # SBUF bf16, and it keeps the mask multiply off
                            # the heavily-loaded DVE queue.
                            nc.gpsimd.tensor_mul(et[:, 0:128], et[:, 0:128],
                                                 trimask[:])
                        ets[j].setdefault(kj, []).append((p0, p1, et))

                def emit_scores_step(b, kj, ets):
                    # Batch 0 starts before phase 1b has produced q/k
                    # columns 1024:2048, so its kj < 3 second pieces are
                    # deferred to step kj+8 (phase 1b completes during
                    # iteration ~3, so later steps emit both pieces
                    # in-step and the quarter bursts never wait on
                    # end-of-stream exps). Batch 1 has all inputs ready.
                    s_lo = 128 * kj
                    defer = (b == 0)
                    if kj < 6 and defer:
                        emit_piece_pair(b, kj, s_lo, 1024, ets)
                    elif kj < 8:
                        emit_piece_pair(b, kj, s_lo, 1024, ets)
                        emit_piece_pair(b, kj, 1024, 2048, ets)
                    else:
                        emit_piece_pair(b, kj, s_lo, 2048, ets)
                        if defer and kj - 8 < 6:
                            emit_piece_pair(b, kj - 8, 1024, 2048, ets)

                def emit_quarter(b, j, q, ets_j):
                    kj_last = 4 * q + 3
                    pq = sh_pool.tile([128, 512], f32, tag="sh", name="pq")
                    for k2 in range(kj_last + 1):
                        a0 = max(512 * q, 128 * k2)
                        a1 = 512 * (q + 1)
                        for p0, p1, et in ets_j[k2]:
                            if p0 <= a0 < p1:
                                break
                        else:
                            raise AssertionError("no piece")
                        nc.tensor.matmul(
                            pq[:, a0 - 512 * q : a1 - 512 * q],
                            lhsT=V_sb[:, NT * b + k2, j, :],
                            rhs=et[:, a0 - p0 : a1 - p0],
                            start=(k2 == 0),
                            stop=(k2 == kj_last),
                        )
                    return pq

                def emit_normalize(b, j, q, pq):
                    # o^T[k, s] / denom[s] for quarter q. Fast-reciprocal
                    # the denominator row (PSUM partition 0), cast to bf16,
                    # broadcast across 64 partitions via a single-pass bf16
                    # K=1 matmul, copy the numerator to SBUF (frees the
                    # burst slot), then one one-PSUM-operand multiply into
                    # OT.
                    rc = rc_pool.tile([1, 512], f32, tag="rc", name="rc")
                    nc.vector.reciprocal_approx_fast(rc[:], pq[0:1, :])
                    rcb = rc_pool.tile([1, 512], bf16, tag="rcb", name="rcb")
                    nc.vector.tensor_copy(rcb[:], rc[:])
                    onum = on_pool.tile([64, 512], f32, tag="onum", name="onum")
                    nc.vector.tensor_copy(onum[:], pq[HD : 2 * HD, :])
                    pb = sh_pool.tile([64, 512], f32, tag="sh", name="pb")
                    nc.tensor.matmul(pb[:], lhsT=ones64[:], rhs=rcb[:],
                                     start=True, stop=True)
                    nc.vector.tensor_mul(
                        OT[64 * j : 64 * (j + 1),
                           S * b + 512 * q : S * b + 512 * (q + 1)],
                        onum[:],
                        pb[:],
                    )

                out_r = out_d.rearrange("(o p) s -> p o s", p=128)

                def emit_v(g):
                    # V for s-block g (both heads side by side): lhsT = xT
                    # s-block (stationary), rhs = wv.
                    pv = sh_pool.tile([128, 128], f32, tag="sh", name="pv")
                    for o in range(DC):
                        nc.tensor.matmul(
                            pv[:],
                            lhsT=xT[:, o, 128 * g : 128 * (g + 1)],
                            rhs=wv[:, o, :],
                            start=(o == 0),
                            stop=(o == DC - 1),
                        )
                    nc.vector.tensor_copy(
                        V_sb[:, g, :, HD : 2 * HD],
                        pv[:].rearrange("p (j k) -> p j k", j=HL),
                    )

                # Projection filler ticks: each (w_sb, dst, sc) unit is two
                # 4-matmul ticks sharing one PSUM tile so a single tick
                # never head-of-line-blocks the latency-critical score
                # matmuls for more than ~1us.
                pending_qk = {}

                def emit_qk_tick(w_sb, dst, sc, half):
                    key = (id(w_sb), sc)
                    if half == 0:
                        pending_qk[key] = sh_pool.tile([128, 512], f32,
                                                       tag="sh", name="pk")
                    pk = pending_qk[key]
                    for o in range(4 * half, 4 * half + 4):
                        nc.tensor.matmul(
                            pk[:],
                            lhsT=w_sb[:, o, :],
                            rhs=xT[:, o, 512 * sc : 512 * (sc + 1)],
                            start=(o == 0),
                            stop=(o == DC - 1),
                        )
                    if half == 1:
                        nc.vector.tensor_copy(dst[:, 512 * sc : 512 * (sc + 1)],
                                              pk[:])
                        del pending_qk[key]

                filler_q = []
                # Phase 1b: batch-0 projection columns 1024:2048 (needed
                # from step 8 / the deferred pieces onward).
                for sc in range(2, 4):
                    for w_sb, dst in ((wq, qT), (wk, kT)):
                        filler_q.append(("qk", w_sb, dst, sc, 0))
                        filler_q.append(("qk", w_sb, dst, sc, 1))
                # batch-0 V blocks
                filler_q += [("v", g) for g in range(16)]
                # batch-1 projections
                for sc in range(4, 8):
                    for w_sb, dst in ((wq, qT), (wk, kT)):
                        filler_q.append(("qk", w_sb, dst, sc, 0))
                        filler_q.append(("qk", w_sb, dst, sc, 1))


                def emit_filler():
                    if not filler_q:
                        return
                    u = filler_q.pop(0)
                    if u[0] == "qk":
                        emit_qk_tick(u[1], u[2], u[3], u[4])
                    else:
                        emit_v(u[1])

                ph4_queue = []      # (b, nb) 512-col chunks awaiting emission
                ph4_state = None    # (b, nb, stage, next_dc)

                def emit_ph4_step(cast_engine="vector", tail=False):
                    nonlocal ph4_state
                    if ph4_state is None:
                        if not ph4_queue:
                            return False
                        b4, nb4 = ph4_queue.pop(0)
                        ph4_state = (b4, nb4, None, 0)
                    b4, nb4, stage, dc = ph4_state
                    if dc % 4 == 0:
                        stage = ph4o.tile([128, 4, 512], bf16, tag="o4", name="o4")
                    # In the tail the score/exp stream is over, so the
                    # 6-bank ps pool is free: alternating pools gives a
                    # 5-slot MM->cast pipeline instead of 2.
                    if tail and dc % 2 == 0:
                        pp4 = ps_pool.tile([128, 1024], f32, tag="ps",
                                           name="pp4t")[:, 0:512]
                    else:
                        pp4 = sh_pool.tile([128, 512], f32, tag="sh", name="pp4")
                    nc.tensor.matmul(
                        pp4,
                        lhsT=wo[:, 128 * dc : 128 * (dc + 1)],
                        rhs=OT[:, S * b4 + 512 * nb4 : S * b4 + 512 * (nb4 + 1)],
                        start=True,
                        stop=True,
                    )
                    if cast_engine == "scalar":
                        nc.scalar.copy(stage[:, dc % 4, :], pp4)
                    else:
                        nc.vector.tensor_copy(stage[:, dc % 4, :], pp4)
                    if dc % 4 == 3:
                        nc.sync.dma_start(
                            out_r[:, dc - 3 : dc + 1,
                                  S * b4 + 512 * nb4 : S * b4 + 512 * (nb4 + 1)],
                            stage[:],
                        )
                    ph4_state = None if dc == DC - 1 else (b4, nb4, stage, dc + 1)
                    return True

                vb1_q = [("v", g) for g in range(16, 32)]

                def emit_backlog(b, kj, ets):
                    # Work scheduled against step kj, emitted AFTER the
                    # NEXT step's scores so the exp stream never waits
                    # behind burst/filler streams on the PE queue.
                    if kj % 4 == 0 and kj > 0:
                        q = kj // 4 - 1
                        for j in range(HL):
                            pq = emit_quarter(b, j, q, ets[j])
                            emit_normalize(b, j, q, pq)
                        ph4_queue.append((b, q))
                    if kj == NT - 1:
                        for j in range(HL):
                            pq = emit_quarter(b, j, 3, ets[j])
                            emit_normalize(b, j, 3, pq)
                        ph4_queue.append((b, 3))
                    if b == 0:
                        # frontload fillers into the early steps where the
                        # exp stream is widest (more PE headroom per step)
                        for _ in range(4 if kj < 8 else 2):
                            emit_filler()
                        emit_ph4_step()
                    else:
                        for _ in range(2):
                            if vb1_q:
                                emit_v(vb1_q.pop(0)[1])
                        # keep the shared-PSUM ring light near the end of
                        # the exp stream: batch-1 units beyond (1,0) drain
                        # in the tail loop with both cast engines and the
                        # freed score-PSUM banks
                        n = 3 if vb1_q else 4
                        for _ in range(n):
                            if ph4_state is None and ph4_queue and \
                                    ph4_queue[0] > (1, 1):
                                break
                            emit_ph4_step()

                # Flat software pipeline over both batches: step i's scores
                # are emitted before step i-1's backlog, including across
                # the pair boundary, so batch 0's final bursts overlap
                # batch 1's first exps.
                steps = [(b, kj) for b in range(B) for kj in range(NT)]
                ets_all = {b: {j: {} for j in range(HL)} for b in range(B)}
                for idx, (b, kj) in enumerate(steps):
                    if (b, kj) == (1, 0):
                        # batch-1 q/k/V inputs must be complete before its
                        # first score matmuls are emitted
                        while filler_q:
                            emit_filler()
                    emit_scores_step(b, kj, ets_all[b])
                    if idx >= 1:
                        pb_, pkj = steps[idx - 1]
                        emit_backlog(pb_, pkj, ets_all[pb_])
                emit_backlog(1, NT - 1, ets_all[1])

                # tail: drain remaining projection chunks with a deep PSUM
                # pipeline and both cast engines so nothing serializes.
                i = 0
                while ph4_queue or ph4_state is not None:
                    if not emit_ph4_step("scalar" if i % 2 == 0 else "vector",
                                         tail=True):
                        break
                    i += 1

    nc.compile()
    return nc


def get_nc():
    if "nc" not in _CACHE:
        _CACHE["nc"] = _build_kernel()
    return _CACHE["nc"]


def make_in_maps(x, Wq, Wk, Wv, Wo):
    """Host-side sharding: per-core input dict (numpy, bf16)."""
    x = np.asarray(x, np.float32)
    Wq = np.asarray(Wq, np.float32)
    Wk = np.asarray(Wk, np.float32)
    Wv = np.asarray(Wv, np.float32)
    Wo = np.asarray(Wo, np.float32)
    xT = np.ascontiguousarray(x.transpose(2, 0, 1).reshape(D, BS)).astype(BF16)
    in_maps = []
    for c in range(NCORES):
        h0 = HL * c

        def pack(W):
            # [HL, D, HD] -> [D, HL*HD]
            return np.ascontiguousarray(
                W[h0 : h0 + HL].transpose(1, 0, 2).reshape(D, HL * HD)
            ).astype(BF16)

        in_maps.append(
            {
                "xT": xT,
                "wq": pack(Wq),
                "wk": pack(Wk),
                "wv": pack(Wv),
                "wo": np.ascontiguousarray(Wo[128 * c : 128 * (c + 1), :]).astype(BF16),
                "consts": _make_consts(),
            }
        )
    return in_maps


def _make_consts():
    if "consts" not in _CACHE:
        tri = (np.arange(128)[None, :] >= np.arange(128)[:, None]).astype(np.float32)
        eye = np.eye(64, dtype=np.float32)
        c = np.zeros((128, 192), np.float32)
        c[:, 0:128] = tri
        c[0:64, 128:192] = eye
        c[64:128, 128:192] = eye
        _CACHE["consts"] = c.astype(BF16)
    return _CACHE["consts"]


def combine_partials(partials, bo):
    acc = np.zeros((D, BS), np.float32)
    for p in partials:
        acc += np.asarray(p, np.float32)
    out = acc.reshape(D, B, S).transpose(1, 2, 0) + np.asarray(bo, np.float32)[None, None, :]
    return np.ascontiguousarray(out.astype(np.float32))


def kernel(x, Wq, Wk, Wv, Wo, bo):
    from concourse.bass_utils import run_bass_kernel_spmd

    nc = get_nc()
    in_maps = make_in_maps(x, Wq, Wk, Wv, Wo)
    res = run_bass_kernel_spmd(nc, in_maps, core_ids=list(range(NCORES)))
    partials = [r["out_pT"] for r in res.results]
    return combine_partials(partials, bo)


# revision 35
# speedup vs baseline: 1.0205x; 1.0205x over previous
"""Multi-head causal attention (B=2, S=2048, D=1024, H=16) on 8 TRN2 cores.

Sharding: tensor-parallel over heads. Core c owns heads {2c, 2c+1} and rows
[128c, 128c+128) of Wo. Each core computes its heads' attention and the
partial output projection; the host sums the 8 partials (the "all-reduce")
and adds the bias.

Device layout (all bf16 in SBUF, f32 PSUM accumulation):
  xT      [1024, 4096]  x transposed: xT[d, b*2048+s] = x[b,s,d]
  wq/wk/wv [1024, 128]  two heads' weights packed on columns
  wo      [128, 1024]   Wo rows for this core
  out_pT  [1024, 4096]  partial^T: out_pT[d, b*2048+s]

Schedule per core:
  1. DMA priority: wq/wk, then batch-0 first-quarter columns of every xT
     d-chunk, then the rest of batch 0, then wv/wo, then batch 1. Phase 1a
     projects Q^T/K^T for batch-0 columns 0:1024 only (DMA-paced); the
     remaining projection chunks (batch-0 second half + batch 1) run as
     4-matmul filler ticks inside the attention loop.
  2. Causal attention per batch with BOTH local heads together. Each kj
     step emits only the (s_lo, 1024) score piece during kj<8; the
     (1024, 2048) pieces are deferred to steps 8..15 (paired with those
     steps' single pieces), so attention starts as soon as phase 1a ends.
     Score matmuls are K=64 and the two heads' kT slices sit at partitions
     0:64 / 64:128 -> different PE row groups; with a 3-deep ps ring both
     heads' matmuls are ready together and stream concurrently. exp on ACT
     (the phase pacer); scores for step kj+1 are emitted BEFORE step kj's
     AV-burst/filler backlog so ACT never starves behind filler streams.
  3. Quarter-major deferred AV per head (one PSUM bank per quarter burst,
     delayed one extra step for pipeline slack); normalization = fast DVE
     reciprocal + bf16 K=1 broadcast matmul + one DVE multiply into OT.
  4. Partial projection out_pT = wo_rows.T @ OT paced as PE filler during
     batch 1's attention; 4-dc half-stages DMA out as soon as they cast.
"""

import numpy as np
import ml_dtypes

B, S, D, H = 2, 2048, 1024, 16
HD = 64          # head dim
NCORES = 8
HL = H // NCORES  # local heads per core = 2
BS = B * S        # 4096
SCALE = float(D) ** -0.5

BF16 = ml_dtypes.bfloat16

_CACHE = {}


def _build_kernel():
    import concourse.mybir as mybir
    import concourse.tile as tile
    from concourse import bacc

    bf16 = mybir.dt.bfloat16
    f32 = mybir.dt.float32
    Exp = mybir.ActivationFunctionType.Exp

    nc = bacc.Bacc("TRN2", debug=False, enable_asserts=False)
    xT_d = nc.dram_tensor("xT", [D, BS], bf16, kind="ExternalInput").ap()
    wq_d = nc.dram_tensor("wq", [D, 128], bf16, kind="ExternalInput").ap()
    wk_d = nc.dram_tensor("wk", [D, 128], bf16, kind="ExternalInput").ap()
    wv_d = nc.dram_tensor("wv", [D, 128], bf16, kind="ExternalInput").ap()
    wo_d = nc.dram_tensor("wo", [128, D], bf16, kind="ExternalInput").ap()
    # consts: cols 0:128 = upper-tri mask (1 where col >= row); cols 128:192
    # unused (kept for host-side layout compat).
    consts_d = nc.dram_tensor("consts", [128, 192], bf16, kind="ExternalInput").ap()
    out_d = nc.dram_tensor("out_pT", [D, BS], bf16, kind="ExternalOutput").ap()

    DC = D // 128   # 8 d-chunks
    NT = S // 128   # 16 key blocks per sequence

    with tile.TileContext(nc) as tc:
        with tc.tile_pool(name="persist", bufs=1) as pp:
            xT = pp.tile([128, DC, BS], bf16, tag="xT")
            qT = pp.tile([128, BS], bf16, tag="qT")
            kT = pp.tile([128, BS], bf16, tag="kT")
            # V in [t, k] layout, padded to 128 columns: col 0 = 1.0 (makes
            # the attention matmul emit softmax denominators in PSUM
            # partition 0), cols 1:64 = 0, cols 64:128 = V block for s-block
            # g (g = 16*b + t16) and local head j. The V block starts at 64
            # so the 64 numerator rows sit at a size-aligned PSUM offset.
            V_sb = pp.tile([128, BS // 128, HL, 128], bf16, tag="V")
            OT = pp.tile([128, BS], bf16, tag="OT")
            wq = pp.tile([128, DC, 128], bf16, tag="wq")
            wk = pp.tile([128, DC, 128], bf16, tag="wk")
            wv = pp.tile([128, DC, 128], bf16, tag="wv")
            wo = pp.tile([128, D], bf16, tag="wo")
            consts = pp.tile([128, 192], bf16, tag="consts")
            trimask = consts[:, 0:128]
            ones64 = pp.tile([1, 64], bf16, tag="ones64")

            nc.sync.dma_start(consts[:], consts_d[:])
            nc.vector.memset(ones64[:], 1.0)
            # Preheat the ACT exp table so the first real exp doesn't pay
            # the table-load latency mid-pipeline.
            warmup = pp.tile([1, 8], bf16, tag="warmup")
            nc.scalar.activation(warmup[:], consts[0:1, 0:8], Exp, scale=SCALE)
            nc.vector.memset(V_sb[:, :, :, 0:HD], 0.0)
            nc.vector.memset(V_sb[:, :, :, 0:1], 1.0)

            # DMA priority order (see module docstring).
            xT_r = xT_d.rearrange("(o p) s -> p o s", p=128)
            nc.sync.dma_start(wq[:], wq_d.rearrange("(o p) c -> p o c", p=128))
            nc.sync.dma_start(wk[:], wk_d.rearrange("(o p) c -> p o c", p=128))
            # per-chunk first-quarter DMAs spread across queues so phase 1a
            # can start on chunk 0 while later chunks stream
            for o in range(DC):
                nc.sync.dma_start(xT[:, o, 0:1024], xT_r[:, o, 0:1024])
            for o in range(0, DC, 4):
                nc.sync.dma_start(xT[:, o : o + 4, 1024:2048],
                                  xT_r[:, o : o + 4, 1024:2048])
            nc.sync.dma_start(wv[:], wv_d.rearrange("(o p) c -> p o c", p=128))
            nc.sync.dma_start(wo[:], wo_d[:])
            for o in range(0, DC, 4):
                nc.sync.dma_start(xT[:, o : o + 4, S:BS],
                                  xT_r[:, o : o + 4, S:BS])

            # ---- Phase 1a: Q^T/K^T for batch-0 columns 0:1024 ----
            # The preheat pool coexists with phase 1a's banks (no WAR), so
            # ~32 dummy matmuls run in phase 1a's DMA-wait gaps and flip
            # the HAM clock gate to 8/8 before the real stream arrives.
            with (
                tc.tile_pool(name="preheat", bufs=1, space="PSUM") as php,
                tc.tile_pool(name="ph1psum", bufs=4, space="PSUM") as ph1,
            ):
                pre = php.tile([128, 512], f32, tag="pre")
                for _ in range(32):
                    nc.tensor.matmul(
                        pre[:, 0:128], lhsT=consts[:, 0:128],
                        rhs=consts[:, 0:128], start=True, stop=True,
                    )
                ps1 = {}
                for di in range(2):
                    for sc in range(2):
                        ps1[(di, sc)] = ph1.tile([128, 512], f32, tag="ph1",
                                                 name=f"ph1_{di}_{sc}")
                for o in range(DC):
                    for di, w_sb in ((0, wq), (1, wk)):
                        for sc in range(2):
                            nc.tensor.matmul(
                                ps1[(di, sc)][:],
                                lhsT=w_sb[:, o, :],
                                rhs=xT[:, o, 512 * sc : 512 * (sc + 1)],
                                start=(o == 0),
                                stop=(o == DC - 1),
                            )
                # copy order q-sc0, k-sc0 first (unblocks the first score
                # matmul), alternating DVE/ACT so the casts overlap
                for n, (di, sc) in enumerate([(0, 0), (1, 0), (0, 1), (1, 1)]):
                    dst = qT if di == 0 else kT
                    t = ps1[(di, sc)]
                    if n % 2 == 0:
                        nc.vector.tensor_copy(dst[:, 512 * sc : 512 * (sc + 1)],
                                              t[:])
                    else:
                        nc.scalar.copy(dst[:, 512 * sc : 512 * (sc + 1)], t[:])

            # ---- Phase 2+3: paired-head causal attention ----
            with (
                tc.tile_pool(name="ps", bufs=3, space="PSUM") as ps_pool,
                tc.tile_pool(name="shared", bufs=2, space="PSUM") as sh_pool,
                tc.tile_pool(name="expp", bufs=36) as exp_pool,
                tc.tile_pool(name="recip", bufs=3) as rc_pool,
                tc.tile_pool(name="onum", bufs=3) as on_pool,
                tc.tile_pool(name="ph4out", bufs=2) as ph4o,
            ):
                def emit_piece_pair(b, kj, p0, p1, ets):
                    # One score piece (columns p0:p1) + exp for BOTH heads.
                    # Head j's matmuls use kT partitions 64j:64j+64 -> PE
                    # row group j; with the 3-deep ps ring both heads'
                    # matmuls are ready together and stream concurrently.
                    s_lo = 128 * kj
                    w = p1 - p0
                    psj = [ps_pool.tile([128, 1024], f32, tag="ps", name="ps")
                           for _ in range(HL)]
                    for c0 in range(0, w, 512):
                        c1 = min(c0 + 512, w)
                        for j in range(HL):
                            kTh = kT[64 * j : 64 * (j + 1), S * b : S * (b + 1)]
                            qTh = qT[64 * j : 64 * (j + 1), S * b : S * (b + 1)]
                            nc.tensor.matmul(
                                psj[j][:, c0:c1],
                                lhsT=kTh[:, s_lo : s_lo + 128],
                                rhs=qTh[:, p0 + c0 : p0 + c1],
                                start=True,
                                stop=True,
                            )
                    for j in range(HL):
                        et = exp_pool.tile([128, 1024], bf16, tag="expT",
                                           name="et")
                        nc.scalar.activation(et[:, 0:w], psj[j][:, 0:w],
                                             Exp, scale=SCALE)
                        if p0 == s_lo:
                            # diagonal 128x128: keep only s' >= t. On
                            # GpSimd (otherwise idle): both operands are
                            # SBUF bf16, and it keeps the mask multiply off
                            # the heavily-loaded DVE queue.
                            nc.gpsimd.tensor_mul(et[:, 0:128], et[:, 0:128],
                                                 trimask[:])
                        ets[j].setdefault(kj, []).append((p0, p1, et))

                def emit_scores_step(b, kj, ets):
                    # Batch 0 starts before phase 1b has produced q/k
                    # columns 1024:2048, so its kj < 3 second pieces are
                    # deferred to step kj+8 (phase 1b completes during
                    # iteration ~3, so later steps emit both pieces
                    # in-step and the quarter bursts never wait on
                    # end-of-stream exps). Batch 1 has all inputs ready.
                    s_lo = 128 * kj
                    defer = (b == 0)
                    if kj < 6 and defer:
                        emit_piece_pair(b, kj, s_lo, 1024, ets)
                    elif kj < 8:
                        emit_piece_pair(b, kj, s_lo, 1024, ets)
                        emit_piece_pair(b, kj, 1024, 2048, ets)
                    else:
                        emit_piece_pair(b, kj, s_lo, 2048, ets)
                        if defer and kj - 8 < 6:
                            emit_piece_pair(b, kj - 8, 1024, 2048, ets)

                def emit_quarter(b, j, q, ets_j):
                    kj_last = 4 * q + 3
                    pq = sh_pool.tile([128, 512], f32, tag="sh", name="pq")
                    for k2 in range(kj_last + 1):
                        a0 = max(512 * q, 128 * k2)
                        a1 = 512 * (q + 1)
                        for p0, p1, et in ets_j[k2]:
                            if p0 <= a0 < p1:
                                break
                        else:
                            raise AssertionError("no piece")
                        nc.tensor.matmul(
                            pq[:, a0 - 512 * q : a1 - 512 * q],
                            lhsT=V_sb[:, NT * b + k2, j, :],
                            rhs=et[:, a0 - p0 : a1 - p0],
                            start=(k2 == 0),
                            stop=(k2 == kj_last),
                        )
                    return pq

                def emit_normalize(b, j, q, pq):
                    # o^T[k, s] / denom[s] for quarter q. Fast-reciprocal
                    # the denominator row (PSUM partition 0), cast to bf16,
                    # broadcast across 64 partitions via a single-pass bf16
                    # K=1 matmul, copy the numerator to SBUF (frees the
                    # burst slot), then one one-PSUM-operand multiply into
                    # OT.
                    rc = rc_pool.tile([1, 512], f32, tag="rc", name="rc")
                    nc.vector.reciprocal_approx_fast(rc[:], pq[0:1, :])
                    rcb = rc_pool.tile([1, 512], bf16, tag="rcb", name="rcb")
                    nc.vector.tensor_copy(rcb[:], rc[:])
                    onum = on_pool.tile([64, 512], f32, tag="onum", name="onum")
                    nc.vector.tensor_copy(onum[:], pq[HD : 2 * HD, :])
                    pb = sh_pool.tile([64, 512], f32, tag="sh", name="pb")
                    nc.tensor.matmul(pb[:], lhsT=ones64[:], rhs=rcb[:],
                                     start=True, stop=True)
                    nc.vector.tensor_mul(
                        OT[64 * j : 64 * (j + 1),
                           S * b + 512 * q : S * b + 512 * (q + 1)],
                        onum[:],
                        pb[:],
                    )

                out_r = out_d.rearrange("(o p) s -> p o s", p=128)

                def emit_v(g):
                    # V for s-block g (both heads side by side): lhsT = xT
                    # s-block (stationary), rhs = wv.
                    pv = sh_pool.tile([128, 128], f32, tag="sh", name="pv")
                    for o in range(DC):
                        nc.tensor.matmul(
                            pv[:],
                            lhsT=xT[:, o, 128 * g : 128 * (g + 1)],
                            rhs=wv[:, o, :],
                            start=(o == 0),
                            stop=(o == DC - 1),
                        )
                    nc.vector.tensor_copy(
                        V_sb[:, g, :, HD : 2 * HD],
                        pv[:].rearrange("p (j k) -> p j k", j=HL),
                    )

                # Projection filler ticks: each (w_sb, dst, sc) unit is two
                # 4-matmul ticks sharing one PSUM tile so a single tick
                # never head-of-line-blocks the latency-critical score
                # matmuls for more than ~1us.
                pending_qk = {}

                def emit_qk_tick(w_sb, dst, sc, half):
                    key = (id(w_sb), sc)
                    if half == 0:
                        pending_qk[key] = sh_pool.tile([128, 512], f32,
                                                       tag="sh", name="pk")
                    pk = pending_qk[key]
                    for o in range(4 * half, 4 * half + 4):
                        nc.tensor.matmul(
                            pk[:],
                            lhsT=w_sb[:, o, :],
                            rhs=xT[:, o, 512 * sc : 512 * (sc + 1)],
                            start=(o == 0),
                            stop=(o == DC - 1),
                        )
                    if half == 1:
                        nc.vector.tensor_copy(dst[:, 512 * sc : 512 * (sc + 1)],
                                              pk[:])
                        del pending_qk[key]

                filler_q = []
                # Phase 1b: batch-0 projection columns 1024:2048 (needed
                # from step 8 / the deferred pieces onward).
                for sc in range(2, 4):
                    for w_sb, dst in ((wq, qT), (wk, kT)):
                        filler_q.append(("qk", w_sb, dst, sc, 0))
                        filler_q.append(("qk", w_sb, dst, sc, 1))
                # batch-0 V blocks
                filler_q += [("v", g) for g in range(16)]
                # batch-1 projections
                for sc in range(4, 8):
                    for w_sb, dst in ((wq, qT), (wk, kT)):
                        filler_q.append(("qk", w_sb, dst, sc, 0))
                        filler_q.append(("qk", w_sb, dst, sc, 1))


                def emit_filler():
                    if not filler_q:
                        return
                    u = filler_q.pop(0)
                    if u[0] == "qk":
                        emit_qk_tick(u[1], u[2], u[3], u[4])
                    else:
                        emit_v(u[1])

                ph4_queue = []      # (b, nb) 512-col chunks awaiting emission
                ph4_state = None    # (b, nb, stage, next_dc)

                def emit_ph4_step(cast_engine="vector", tail=False):
                    nonlocal ph4_state
                    if ph4_state is None:
                        if not ph4_queue:
                            return False
                        b4, nb4 = ph4_queue.pop(0)
                        ph4_state = (b4, nb4, None, 0)
                    b4, nb4, stage, dc = ph4_state
                    if dc % 4 == 0:
                        stage = ph4o.tile([128, 4, 512], bf16, tag="o4", name="o4")
                    # In the tail the score/exp stream is over, so the
                    # 6-bank ps pool is free: alternating pools gives a
                    # 5-slot MM->cast pipeline instead of 2.
                    if tail and dc % 2 == 0:
                        pp4 = ps_pool.tile([128, 1024], f32, tag="ps",
                                           name="pp4t")[:, 0:512]
                    else:
                        pp4 = sh_pool.tile([128, 512], f32, tag="sh", name="pp4")
                    nc.tensor.matmul(
                        pp4,
                        lhsT=wo[:, 128 * dc : 128 * (dc + 1)],
                        rhs=OT[:, S * b4 + 512 * nb4 : S * b4 + 512 * (nb4 + 1)],
                        start=True,
                        stop=True,
                    )
                    if cast_engine == "scalar":
                        nc.scalar.copy(stage[:, dc % 4, :], pp4)
                    else:
                        nc.vector.tensor_copy(stage[:, dc % 4, :], pp4)
                    if dc % 4 == 3:
                        nc.sync.dma_start(
                            out_r[:, dc - 3 : dc + 1,
                                  S * b4 + 512 * nb4 : S * b4 + 512 * (nb4 + 1)],
                            stage[:],
                        )
                    ph4_state = None if dc == DC - 1 else (b4, nb4, stage, dc + 1)
                    return True

                vb1_q = [("v", g) for g in range(16, 32)]

                def emit_backlog(b, kj, ets):
                    # Work scheduled against step kj, emitted AFTER the
                    # NEXT step's scores so the exp stream never waits
                    # behind burst/filler streams on the PE queue.
                    if kj % 4 == 0 and kj > 0:
                        q = kj // 4 - 1
                        for j in range(HL):
                            pq = emit_quarter(b, j, q, ets[j])
                            emit_normalize(b, j, q, pq)
                        ph4_queue.append((b, q))
                    if kj == NT - 1:
                        for j in range(HL):
                            pq = emit_quarter(b, j, 3, ets[j])
                            emit_normalize(b, j, 3, pq)
                        ph4_queue.append((b, 3))
                    if b == 0:
                        emit_filler()
                        emit_filler()
                        emit_filler()
                        emit_ph4_step()
                    else:
                        for _ in range(2):
                            if vb1_q:
                                emit_v(vb1_q.pop(0)[1])
                        # keep the shared-PSUM ring light near the end of
                        # the exp stream: batch-1 units beyond (1,0) drain
                        # in the tail loop with both cast engines and the
                        # freed score-PSUM banks
                        n = 3 if vb1_q else 4
                        for _ in range(n):
                            if ph4_state is None and ph4_queue and \
                                    ph4_queue[0] > (1, 1):
                                break
                            emit_ph4_step()

                # Flat software pipeline over both batches: step i's scores
                # are emitted before step i-1's backlog, including across
                # the pair boundary, so batch 0's final bursts overlap
                # batch 1's first exps.
                steps = [(b, kj) for b in range(B) for kj in range(NT)]
                ets_all = {b: {j: {} for j in range(HL)} for b in range(B)}
                for idx, (b, kj) in enumerate(steps):
                    if (b, kj) == (1, 0):
                        # batch-1 q/k/V inputs must be complete before its
                        # first score matmuls are emitted
                        while filler_q:
                            emit_filler()
                    emit_scores_step(b, kj, ets_all[b])
                    if idx >= 1:
                        pb_, pkj = steps[idx - 1]
                        emit_backlog(pb_, pkj, ets_all[pb_])
                emit_backlog(1, NT - 1, ets_all[1])

                # tail: drain remaining projection chunks with a deep PSUM
                # pipeline and both cast engines so nothing serializes.
                i = 0
                while ph4_queue or ph4_state is not None:
                    if not emit_ph4_step("scalar" if i % 2 == 0 else "vector",
                                         tail=True):
                        break
                    i += 1

    nc.compile()
    return nc


def get_nc():
    if "nc" not in _CACHE:
        _CACHE["nc"] = _build_kernel()
    return _CACHE["nc"]


def make_in_maps(x, Wq, Wk, Wv, Wo):
    """Host-side sharding: per-core input dict (numpy, bf16)."""
    x = np.asarray(x, np.float32)
    Wq = np.asarray(Wq, np.float32)
    Wk = np.asarray(Wk, np.float32)
    Wv = np.asarray(Wv, np.float32)
    Wo = np.asarray(Wo, np.float32)
    xT = np.ascontiguousarray(x.transpose(2, 0, 1).reshape(D, BS)).astype(BF16)
    in_maps = []
    for c in range(NCORES):
        h0 = HL * c

        def pack(W):
            # [HL, D, HD] -> [D, HL*HD]
            return np.ascontiguousarray(
                W[h0 : h0 + HL].transpose(1, 0, 2).reshape(D, HL * HD)
            ).astype(BF16)

        in_maps.append(
            {
                "xT": xT,
                "wq": pack(Wq),
                "wk": pack(Wk),
                "wv": pack(Wv),
                "wo": np.ascontiguousarray(Wo[128 * c : 128 * (c + 1), :]).astype(BF16),
                "consts": _make_consts(),
            }
        )
    return in_maps


def _make_consts():
    if "consts" not in _CACHE:
        tri = (np.arange(128)[None, :] >= np.arange(128)[:, None]).astype(np.float32)
        eye = np.eye(64, dtype=np.float32)
        c = np.zeros((128, 192), np.float32)
        c[:, 0:128] = tri
        c[0:64, 128:192] = eye
        c[64:128, 128:192] = eye
        _CACHE["consts"] = c.astype(BF16)
    return _CACHE["consts"]


def combine_partials(partials, bo):
    acc = np.zeros((D, BS), np.float32)
    for p in partials:
        acc += np.asarray(p, np.float32)
    out = acc.reshape(D, B, S).transpose(1, 2, 0) + np.asarray(bo, np.float32)[None, None, :]
    return np.ascontiguousarray(out.astype(np.float32))


def kernel(x, Wq, Wk, Wv, Wo, bo):
    from concourse.bass_utils import run_bass_kernel_spmd

    nc = get_nc()
    in_maps = make_in_maps(x, Wq, Wk, Wv, Wo)
    res = run_bass_kernel_spmd(nc, in_maps, core_ids=list(range(NCORES)))
    partials = [r["out_pT"] for r in res.results]
    return combine_partials(partials, bo)
